# revision 9
# baseline (speedup 1.0000x reference)
"""RWKV single-token inference kernel for 8 trn2 NeuronCores.

Strategy: tensor-parallel over 8 cores.
 - All activations replicated on every core in "column" layout [128, 8]
   (tile[p, j] = vec[j*128 + p]).
 - Every matvec output-sharded (rw/kw/vw/frw: 128 rows per core,
   fkw: 512 rows per core) or input-sharded (ow, fvw) so that each layer
   needs exactly TWO small AllReduces:
     AR1: partial ow @ wkv            -> full att output everywhere
     AR2: partial fvw @ fk  ||  fr    -> full ffn output + full fr
   (fr is gathered by zero-masking: core c contributes its column c).
 - Weights are host-pre-transposed into the stationary-operand (lhsT)
   layout so each [128,128] chunk is one LDWEIGHTS+MATMUL pair with the
   activation column [128,1] as the moving operand; matvec outputs land
   column-major across partitions, matching the elementwise layout.
 - Head (50277x1024) output-sharded over V: weight is the *moving*
   operand ([1,512] psum rows), streamed; no collective needed.
LayerNorm stats are computed with ones-vector matmuls (cross-partition
reduce + broadcast on the PE).
"""

import numpy as np

import concourse.bass as bass
import concourse.mybir as mybir
import concourse.tile as tile

L = 24
E = 1024
H = 4 * E
V = 50277
NC = 8
P = 128
JE = E // P          # 8 column-chunks per E-vector
LN_EPS = 1e-5

# per-core head slice (padded)
HV = 6400            # head rows per core (8*6400 = 51200 >= V)
HNT = 13             # 13 tiles of 512 -> 6656 cols of psum output
HPAD = HNT * 512     # 6656

F32 = mybir.dt.float32
BF16 = mybir.dt.bfloat16
WDT = F32  # weight/matvec-operand dtype
AX = mybir.AxisListType
OP = mybir.AluOpType
AF = mybir.ActivationFunctionType

# column offsets inside the per-layer weight block [128, 13312]
WRW, WKW, WVW, WOW, WFRW, WFKW, WFVW = (
    0, 1024, 2048, 3072, 4096, 5120, 9216)
WCOLS = 13312

# per-layer vec columns (all [128,8] blocks except tf/td shards)
_VNAMES = [
    ("h_k", 8), ("h_v", 8), ("h_r", 8),
    ("om_tmk", 8), ("om_tmv", 8), ("om_tmr", 8),
    ("b_tmk", 8), ("b_tmv", 8), ("b_tmr", 8),
    ("l1w", 8), ("l1b", 8),
    ("h_fk", 8), ("h_fr", 8),
    ("om_ftmk", 8), ("om_ftmr", 8),
    ("b_ftmk", 8), ("b_ftmr", 8),
    ("l2w", 8), ("l2b", 8),
    ("tf", 1), ("td", 1),
]
VOFF = {}
_c = 0
for _n, _w in _VNAMES:
    VOFF[_n] = _c
    _c += _w
VCOLS = _c  # 154

SCOLS = 19  # state in/out per layer: attx(8) ffnx(8) aa bb pp / xn(8) xn2(8) naa nbb npp

RG = [list(range(NC))]


def _build_nc(nl=L):
    nc = bass.Bass(trn_type="TRN2", num_devices=NC)

    wall_d = nc.dram_tensor("wall", [nl, P, WCOLS], WDT, kind="ExternalInput")
    hrhs_d = nc.dram_tensor("hrhs", [HNT, P, 8 * 512], WDT, kind="ExternalInput")
    vecs_d = nc.dram_tensor("vecs", [P, VCOLS * nl], F32, kind="ExternalInput")
    stin_d = nc.dram_tensor("stin", [P, SCOLS * nl], F32, kind="ExternalInput")
    xin_d = nc.dram_tensor("xin", [P, JE], F32, kind="ExternalInput")
    gvec_d = nc.dram_tensor("gvec", [P, 16], F32, kind="ExternalInput")
    mask_d = nc.dram_tensor("mask", [P, JE], F32, kind="ExternalInput")

    logits_d = nc.dram_tensor("logits", [1, HPAD], F32, kind="ExternalOutput")
    stout_d = nc.dram_tensor("stout", [P, SCOLS * nl], F32, kind="ExternalOutput")

    with tile.TileContext(nc, num_cores=NC) as tc:
        with (
            tc.tile_pool(name="res", bufs=1) as res,
            tc.tile_pool(name="wpool", bufs=2) as wpool,
            tc.tile_pool(name="hpool", bufs=2) as hpool,
            tc.tile_pool(name="sc", bufs=2) as sc,
            tc.tile_pool(name="ps8", bufs=2, space="PSUM") as ps8,
            tc.tile_pool(name="psmv", bufs=4, space="PSUM") as psmv,
            tc.tile_pool(name="psln", bufs=1, space="PSUM") as psln,
            tc.tile_pool(name="psbc", bufs=1, space="PSUM") as psbc,
            tc.tile_pool(name="dpool", bufs=2, space="DRAM") as dpool,
        ):
            # ---- resident tiles ----
            vecs = res.tile([P, VCOLS * nl], F32, tag="vecs")
            stin = res.tile([P, SCOLS * nl], F32, tag="stin")
            sout = res.tile([P, SCOLS * nl], F32, tag="sout")
            xin = res.tile([P, JE], F32, tag="xin")
            gvec = res.tile([P, 16], F32, tag="gvec")
            mask = res.tile([P, JE], F32, tag="mask")
            ones_c = res.tile([P, 1], F32, tag="ones_c")
            ones_r = res.tile([1, P], F32, tag="ones_r")
            epsc = res.tile([1, 1], F32, tag="epsc")

            nc.sync.dma_start(vecs[:], vecs_d[:])
            nc.sync.dma_start(stin[:], stin_d[:])
            nc.sync.dma_start(xin[:], xin_d[:])
            nc.sync.dma_start(gvec[:], gvec_d[:])
            nc.sync.dma_start(mask[:], mask_d[:])
            nc.vector.memset(ones_c[:], 1.0)
            nc.vector.memset(ones_r[:], 1.0)
            nc.vector.memset(epsc[:], LN_EPS)

            def vec(l, name, w=8):
                o = l * VCOLS + VOFF[name]
                return vecs[:, o:o + w]

            def ln_z(x_ap, ztag, zdt=F32):
                """z = (x - mean) * rstd, returns [128,8] tile."""
                S = sc.tile([P, 16], F32, tag=ztag + "S")
                nc.vector.tensor_copy(S[:, 0:8], x_ap)
                nc.scalar.square(S[:, 8:16], x_ap)
                lps = psln.tile([1, 16], F32, tag="lnps")
                nc.tensor.matmul(lps[:], ones_c[:], S[:], start=True, stop=True)
                st = sc.tile([1, 2], F32, tag=ztag + "st")
                nc.vector.tensor_reduce(
                    st[:], lps[0:1, :].rearrange("p (g x) -> p g x", g=2),
                    axis=AX.X, op=OP.add)
                nc.vector.tensor_scalar_mul(st[:], st[:], 1.0 / E)
                msq = sc.tile([1, 1], F32, tag=ztag + "m2")
                nc.vector.tensor_mul(msq[:], st[:, 0:1], st[:, 0:1])
                nc.vector.tensor_sub(st[:, 1:2], st[:, 1:2], msq[:])
                nc.scalar.activation(st[:, 1:2], st[:, 1:2], AF.Sqrt,
                                     bias=epsc[:])
                nc.vector.reciprocal(st[:, 1:2], st[:, 1:2])
                bps = psbc.tile([P, 2], F32, tag="bcps")
                nc.tensor.matmul(bps[:], ones_r[:], st[:], start=True, stop=True)
                mb = sc.tile([P, 2], F32, tag=ztag + "mb")
                nc.vector.tensor_copy(mb[:], bps[:])
                z = sc.tile([P, JE], zdt, tag=ztag)
                nc.vector.tensor_scalar(
                    z[:], x_ap, mb[:, 0:1], mb[:, 1:2],
                    op0=OP.subtract, op1=OP.mult)
                return z

            def mix(z, l, hname, gsrc, omname, bname, tag):
                """xk = z*h + (src*om + b); returns [128,8] tile."""
                g = sc.tile([P, JE], F32, tag=tag + "g")
                nc.vector.tensor_mul(g[:], gsrc, vec(l, omname))
                nc.vector.tensor_add(g[:], g[:], vec(l, bname))
                xm = sc.tile([P, JE], WDT, tag=tag)
                nc.vector.tensor_mul(xm[:], z[:], vec(l, hname))
                nc.vector.tensor_add(xm[:], xm[:], g[:])
                return xm

            # ---- ln0 ----
            z0 = ln_z(xin[:], "z0")
            x = sc.tile([P, JE], F32, tag="x", bufs=3)
            nc.vector.tensor_mul(x[:], z0[:], gvec[:, 0:8])
            nc.vector.tensor_add(x[:], x[:], gvec[:, 8:16])

            for l in range(nl):
                so = l * SCOLS
                attx = stin[:, so:so + 8]
                ffnx = stin[:, so + 8:so + 16]
                aa = stin[:, so + 16:so + 17]
                bb = stin[:, so + 17:so + 18]
                pp = stin[:, so + 18:so + 19]

                wsb = wpool.tile([P, WCOLS], WDT, tag="W")
                nc.sync.dma_start(wsb[:], wall_d[l])

                # ---- attention ----
                z1 = ln_z(x[:], "z1")
                # state: xn = z1*l1w + l1b
                nc.vector.tensor_mul(sout[:, so:so + 8], z1[:], vec(l, "l1w"))
                nc.vector.tensor_add(sout[:, so:so + 8], sout[:, so:so + 8],
                                     vec(l, "l1b"))
                xk = mix(z1, l, "h_k", attx, "om_tmk", "b_tmk", "xk")
                xv = mix(z1, l, "h_v", attx, "om_tmv", "b_tmv", "xv")
                xr = mix(z1, l, "h_r", attx, "om_tmr", "b_tmr", "xr")

                kps = psmv.tile([P, 1], F32, tag="mv")
                vps = psmv.tile([P, 1], F32, tag="mv")
                rps = psmv.tile([P, 1], F32, tag="mv")
                for ps, base, xm in ((kps, WKW, xk), (vps, WVW, xv),
                                     (rps, WRW, xr)):
                    for j in range(JE):
                        nc.tensor.matmul(
                            ps[:], wsb[:, base + j * P: base + (j + 1) * P],
                            xm[:, j:j + 1], start=(j == 0), stop=(j == JE - 1))

                k = sc.tile([P, 1], F32, tag="k")
                v = sc.tile([P, 1], F32, tag="v")
                nc.vector.tensor_copy(k[:], kps[:])
                nc.vector.tensor_copy(v[:], vps[:])
                r = sc.tile([P, 1], F32, tag="r")
                nc.scalar.activation(r[:], rps[:], AF.Sigmoid)

                tf = vec(l, "tf", 1)
                td = vec(l, "td", 1)
                ww = sc.tile([P, 1], F32, tag="ww")
                nc.vector.tensor_scalar_add(ww[:], k[:], tf)
                p_ = sc.tile([P, 1], F32, tag="p_")
                nc.vector.tensor_scalar_max(p_[:], ww[:], pp)
                e1 = sc.tile([P, 1], F32, tag="e1")
                nc.scalar.activation(e1[:], p_[:], AF.Exp, bias=pp, scale=-1.0)
                e2 = sc.tile([P, 1], F32, tag="e2")
                nc.scalar.activation(e2[:], p_[:], AF.Exp, bias=ww[:], scale=-1.0)
                t2 = sc.tile([P, 1], F32, tag="t2")
                nc.vector.tensor_scalar_mul(t2[:], e2[:], v[:])
                a_ = sc.tile([P, 1], F32, tag="a_")
                nc.vector.scalar_tensor_tensor(
                    a_[:], e1[:], aa, t2[:], op0=OP.mult, op1=OP.add)
                b_ = sc.tile([P, 1], F32, tag="b_")
                nc.vector.scalar_tensor_tensor(
                    b_[:], e1[:], bb, e2[:], op0=OP.mult, op1=OP.add)
                rec = sc.tile([P, 1], F32, tag="rec")
                nc.vector.reciprocal(rec[:], b_[:])
                wkv = sc.tile([P, 1], WDT, tag="wkv")
                nc.vector.tensor_scalar(
                    wkv[:], a_[:], r[:], rec[:], op0=OP.mult, op1=OP.mult)

                # state update
                ww2 = sc.tile([P, 1], F32, tag="ww2")
                nc.vector.tensor_scalar_add(ww2[:], pp, td)
                npp = sout[:, so + 18:so + 19]
                nc.vector.tensor_scalar_max(npp, ww2[:], k[:])
                f1 = sc.tile([P, 1], F32, tag="f1")
                nc.scalar.activation(f1[:], npp, AF.Exp, bias=ww2[:], scale=-1.0)
                f2 = sc.tile([P, 1], F32, tag="f2")
                nc.scalar.activation(f2[:], npp, AF.Exp, bias=k[:], scale=-1.0)
                t4 = sc.tile([P, 1], F32, tag="t4")
                nc.vector.tensor_scalar_mul(t4[:], f2[:], v[:])
                nc.vector.scalar_tensor_tensor(
                    sout[:, so + 16:so + 17], f1[:], aa, t4[:],
                    op0=OP.mult, op1=OP.add)
                nc.vector.scalar_tensor_tensor(
                    sout[:, so + 17:so + 18], f1[:], bb, f2[:],
                    op0=OP.mult, op1=OP.add)

                # ow partial (input-sharded): y_t = ow[:,c]_tile_t @ wkv
                yps = ps8.tile([P, JE], F32, tag="yz")
                for t in range(JE):
                    nc.tensor.matmul(
                        yps[:, t:t + 1], wsb[:, WOW + t * P: WOW + (t + 1) * P],
                        wkv[:], start=True, stop=True)
                ysb = sc.tile([P, JE], F32, tag="ysb")
                nc.vector.tensor_copy(ysb[:], yps[:])

                b1i = dpool.tile([P, JE], F32, tag="b1i")
                b1o = dpool.tile([P, JE], F32, tag="b1o")
                nc.gpsimd.dma_start(b1i[:], ysb[:])
                nc.gpsimd.collective_compute(
                    "AllReduce", OP.add, replica_groups=RG,
                    ins=[b1i.opt()], outs=[b1o.opt()])
                yf = sc.tile([P, JE], F32, tag="yf")
                nc.gpsimd.dma_start(yf[:], b1o[:])

                sx = sc.tile([P, JE], F32, tag="sx")
                nc.vector.tensor_add(sx[:], x[:], yf[:])

                # ---- ffn ----
                z2 = ln_z(sx[:], "z2")
                nc.vector.tensor_mul(sout[:, so + 8:so + 16], z2[:], vec(l, "l2w"))
                nc.vector.tensor_add(sout[:, so + 8:so + 16],
                                     sout[:, so + 8:so + 16], vec(l, "l2b"))
                fxk = mix(z2, l, "h_fk", ffnx, "om_ftmk", "b_ftmk", "fxk")
                fxr = mix(z2, l, "h_fr", ffnx, "om_ftmr", "b_ftmr", "fxr")

                fkps = psmv.tile([P, 4], F32, tag="mv")
                for t in range(4):
                    for j in range(JE):
                        nc.tensor.matmul(
                            fkps[:, t:t + 1],
                            wsb[:, WFKW + (t * 8 + j) * P: WFKW + (t * 8 + j + 1) * P],
                            fxk[:, j:j + 1], start=(j == 0), stop=(j == JE - 1))
                fkr = sc.tile([P, 4], F32, tag="fkr")
                nc.scalar.activation(fkr[:], fkps[:], AF.Relu)
                fks = sc.tile([P, 4], WDT, tag="fks")
                nc.vector.tensor_mul(fks[:], fkr[:], fkr[:])

                frps = psmv.tile([P, 1], F32, tag="mv")
                for j in range(JE):
                    nc.tensor.matmul(
                        frps[:], wsb[:, WFRW + j * P: WFRW + (j + 1) * P],
                        fxr[:, j:j + 1], start=(j == 0), stop=(j == JE - 1))
                fr = sc.tile([P, 1], F32, tag="fr")
                nc.scalar.activation(fr[:], frps[:], AF.Sigmoid)

                zps = ps8.tile([P, JE], F32, tag="yz")
                for t in range(JE):
                    for j in range(4):
                        nc.tensor.matmul(
                            zps[:, t:t + 1],
                            wsb[:, WFVW + (t * 4 + j) * P: WFVW + (t * 4 + j + 1) * P],
                            fks[:, j:j + 1], start=(j == 0), stop=(j == 3))

                zfr = sc.tile([P, 16], F32, tag="zfr")
                nc.vector.tensor_copy(zfr[:, 0:8], zps[:])
                nc.vector.tensor_scalar_mul(zfr[:, 8:16], mask[:], fr[:])

                b2i = dpool.tile([P, 16], F32, tag="b2i")
                b2o = dpool.tile([P, 16], F32, tag="b2o")
                nc.gpsimd.dma_start(b2i[:], zfr[:])
                nc.gpsimd.collective_compute(
                    "AllReduce", OP.add, replica_groups=RG,
                    ins=[b2i.opt()], outs=[b2o.opt()])
                yf2 = sc.tile([P, 16], F32, tag="yf2")
                nc.gpsimd.dma_start(yf2[:], b2o[:])

                fz = sc.tile([P, JE], F32, tag="fz")
                nc.vector.tensor_mul(fz[:], yf2[:, 0:8], yf2[:, 8:16])
                xn = sc.tile([P, JE], F32, tag="x", bufs=3)
                nc.vector.tensor_add(xn[:], sx[:], fz[:])
                x = xn

            # ---- head ----
            zo = ln_z(x[:], "zo", zdt=WDT)
            for nt in range(HNT):
                hsb = hpool.tile([P, 8 * 512], WDT, tag="H")
                nc.sync.dma_start(hsb[:], hrhs_d[nt])
                hps = ps8.tile([1, 512], F32, tag="yz")
                for j in range(JE):
                    nc.tensor.matmul(
                        hps[:], zo[:, j:j + 1], hsb[:, j * 512:(j + 1) * 512],
                        start=(j == 0), stop=(j == JE - 1))
                lrow = sc.tile([1, 512], F32, tag="lrow")
                if nt % 2 == 0:
                    nc.vector.tensor_copy(lrow[:], hps[:])
                else:
                    nc.scalar.copy(lrow[:], hps[:])
                nc.sync.dma_start(logits_d[0:1, nt * 512:(nt + 1) * 512], lrow[:])

            nc.sync.dma_start(stout_d[:], sout[:])

    return nc


def _split_multi_waits(nc):
    """Walrus codegen only supports ONE sync-wait per instruction; move
    extra waits onto same-engine NoOps inserted just before."""
    n = 0
    for f in nc.m.functions:
        for blk in f.blocks:
            new = []
            for ins in blk.instructions:
                si = ins.sync_info
                if si is not None and si.on_wait is not None \
                        and len(si.on_wait) > 1:
                    waits = list(si.on_wait)
                    for w in waits[:-1]:
                        nop = mybir.InstNoOp(
                            name=f"I-ws-{n}", ins=[], outs=[])
                        n += 1
                        nop.engine = ins.engine
                        nop.sync_info = mybir.SyncInfo(
                            on_wait=[w], on_update=[])
                        new.append(nop)
                    si.on_wait = [waits[-1]]
                new.append(ins)
            blk.instructions = new


# ---------------------------------------------------------------------------
# host-side prep


def _cols(vv):
    """[1024] -> [128, 8] column layout."""
    return np.ascontiguousarray(vv.reshape(JE, P).T)


def _uncols(m):
    """[128, 8] -> [1024]."""
    return np.ascontiguousarray(m.T).reshape(-1)


def _prep(inp, nl=L):
    f = lambda name: np.asarray(inp[name], np.float32)
    emb = f("emb")
    state = f("state").reshape(nl if False else L, 5, E)[:nl]
    token = int(np.asarray(inp["token"]))

    tmk, tmv, tmr = f("att_tmk")[:nl], f("att_tmv")[:nl], f("att_tmr")[:nl]
    tf, td = f("att_tf")[:nl], f("att_td")[:nl]
    l1w, l1b = f("ln1_w")[:nl], f("ln1_b")[:nl]
    l2w, l2b = f("ln2_w")[:nl], f("ln2_b")[:nl]
    ftmk, ftmr = f("ffn_tmk")[:nl], f("ffn_tmr")[:nl]
    kw, vw, rw, ow = f("att_kw")[:nl], f("att_vw")[:nl], f("att_rw")[:nl], f("att_ow")[:nl]
    fkw, fvw, frw = f("ffn_kw")[:nl], f("ffn_vw")[:nl], f("ffn_rw")[:nl]
    lnow, lnob = f("ln_out_w"), f("ln_out_b")
    head = f("head_w")

    x0 = emb[token]

    # head folding
    hw2 = head * lnow[None, :]
    hw2 = np.concatenate(
        [hw2, np.zeros((NC * HV - V, E), np.float32)], axis=0)
    logit0 = head @ lnob

    in_maps = []
    for c in range(NC):
        r0, r1 = c * P, (c + 1) * P
        wall = np.empty((nl, P, WCOLS), np.float32)
        vecs = np.empty((P, VCOLS * nl), np.float32)
        stin = np.empty((P, SCOLS * nl), np.float32)
        for l in range(nl):
            # output-sharded E x E (stationary layout [k, j, m])
            for base, W in ((WRW, rw[l]), (WKW, kw[l]), (WVW, vw[l])):
                A = W[r0:r1, :]                       # [128, 1024]
                arr = A.T.reshape(JE, P, P).transpose(1, 0, 2)
                wall[l, :, base:base + 1024] = arr.reshape(P, 1024)
            # ow input-sharded
            B = ow[l][:, r0:r1]                       # [1024, 128]
            wall[l, :, WOW:WOW + 1024] = B.T.reshape(P, 1024)
            # frw output-sharded
            A = frw[l][r0:r1, :]
            arr = A.T.reshape(JE, P, P).transpose(1, 0, 2)
            wall[l, :, WFRW:WFRW + 1024] = arr.reshape(P, 1024)
            # fkw output-sharded [512, 1024]
            C = fkw[l][c * 512:(c + 1) * 512, :]
            arr = C.T.reshape(JE, P, 4, P).transpose(1, 2, 0, 3)
            wall[l, :, WFKW:WFKW + 4096] = arr.reshape(P, 4096)
            # fvw input-sharded [1024, 512]
            D = fvw[l][:, c * 512:(c + 1) * 512]
            arr = D.T.reshape(4, P, JE, P).transpose(1, 2, 0, 3)
            wall[l, :, WFVW:WFVW + 4096] = arr.reshape(P, 4096)

            o = l * VCOLS
            pairs = {
                "h_k": l1w[l] * tmk[l], "h_v": l1w[l] * tmv[l],
                "h_r": l1w[l] * tmr[l],
                "om_tmk": 1.0 - tmk[l], "om_tmv": 1.0 - tmv[l],
                "om_tmr": 1.0 - tmr[l],
                "b_tmk": l1b[l] * tmk[l], "b_tmv": l1b[l] * tmv[l],
                "b_tmr": l1b[l] * tmr[l],
                "l1w": l1w[l], "l1b": l1b[l],
                "h_fk": l2w[l] * ftmk[l], "h_fr": l2w[l] * ftmr[l],
                "om_ftmk": 1.0 - ftmk[l], "om_ftmr": 1.0 - ftmr[l],
                "b_ftmk": l2b[l] * ftmk[l], "b_ftmr": l2b[l] * ftmr[l],
                "l2w": l2w[l], "l2b": l2b[l],
            }
            for nme, val in pairs.items():
                vecs[:, o + VOFF[nme]:o + VOFF[nme] + 8] = _cols(val)
            vecs[:, o + VOFF["tf"]] = tf[l, r0:r1]
            vecs[:, o + VOFF["td"]] = td[l, r0:r1]

            so = l * SCOLS
            stin[:, so:so + 8] = _cols(state[l, 1])       # att_x
            stin[:, so + 8:so + 16] = _cols(state[l, 0])  # ffn_x
            stin[:, so + 16] = state[l, 2, r0:r1]         # aa
            stin[:, so + 17] = state[l, 3, r0:r1]         # bb
            stin[:, so + 18] = state[l, 4, r0:r1]         # pp

        Hc = hw2[c * HV:(c + 1) * HV]                     # [6400, 1024]
        Hp = np.concatenate(
            [Hc, np.zeros((HPAD - HV, E), np.float32)], axis=0)
        hrhs = Hp.T.reshape(JE, P, HNT, 512).transpose(2, 1, 0, 3)
        hrhs = np.ascontiguousarray(hrhs.reshape(HNT, P, 8 * 512))

        gvec = np.empty((P, 16), np.float32)
        gvec[:, 0:8] = _cols(f("ln0_w"))
        gvec[:, 8:16] = _cols(f("ln0_b"))

        msk = np.zeros((P, JE), np.float32)
        msk[:, c] = 1.0

        in_maps.append({
            "wall": np.ascontiguousarray(wall),
            "hrhs": hrhs,
            "vecs": np.ascontiguousarray(vecs),
            "stin": np.ascontiguousarray(stin),
            "xin": _cols(x0),
            "gvec": gvec,
            "mask": msk,
        })
    return in_maps, logit0


def _assemble(results, logit0, nl=L):
    logits = np.concatenate(
        [results[c]["logits"][0, :HV] for c in range(NC)])[:V] + logit0
    st = results[0]["stout"]
    new_state = np.empty((5 * nl, E), np.float32)
    for l in range(nl):
        so = l * SCOLS
        new_state[5 * l + 0] = _uncols(st[:, so + 8:so + 16])   # ffn_x = xn2
        new_state[5 * l + 1] = _uncols(st[:, so:so + 8])        # att_x = xn
        for c in range(NC):
            stc = results[c]["stout"]
            new_state[5 * l + 2, c * P:(c + 1) * P] = stc[:, so + 16]
            new_state[5 * l + 3, c * P:(c + 1) * P] = stc[:, so + 17]
            new_state[5 * l + 4, c * P:(c + 1) * P] = stc[:, so + 18]
    return logits.astype(np.float32), new_state


_NC_CACHE = {}


def get_nc(nl=L):
    """Build + legalize for the hardware path (sim chokes on the NoOps)."""
    if nl not in _NC_CACHE:
        nc = _build_nc(nl)
        _split_multi_waits(nc)
        _NC_CACHE[nl] = nc
    return _NC_CACHE[nl]


def kernel(**inputs):
    from concourse.bass_utils import run_bass_kernel_spmd

    nc = get_nc(L)
    in_maps, logit0 = _prep(inputs, L)
    res = run_bass_kernel_spmd(nc, in_maps, core_ids=list(range(NC)))
    return _assemble(res.results, logit0, L)


# revision 10
# speedup vs baseline: 1.7889x; 1.7889x over previous
"""RWKV single-token inference kernel for 8 trn2 NeuronCores.

Strategy: tensor-parallel over 8 cores.
 - All activations replicated on every core in "column" layout [128, 8]
   (tile[p, j] = vec[j*128 + p]).
 - Every matvec output-sharded (rw/kw/vw/frw: 128 rows per core,
   fkw: 512 rows per core) or input-sharded (ow, fvw) so that each layer
   needs exactly TWO small AllReduces:
     AR1: partial ow @ wkv            -> full att output everywhere
     AR2: partial fvw @ fk  ||  fr    -> full ffn output + full fr
   (fr is gathered by zero-masking: core c contributes its column c).
 - Weights are host-pre-transposed into the stationary-operand (lhsT)
   layout so each [128,128] chunk is one LDWEIGHTS+MATMUL pair with the
   activation column [128,1] as the moving operand; matvec outputs land
   column-major across partitions, matching the elementwise layout.
 - Head (50277x1024) output-sharded over V: weight is the *moving*
   operand ([1,512] psum rows), streamed; no collective needed.
LayerNorm stats are computed with ones-vector matmuls (cross-partition
reduce + broadcast on the PE).
"""

import numpy as np
import ml_dtypes

import concourse.bass as bass
import concourse.mybir as mybir
import concourse.tile as tile

L = 24
E = 1024
H = 4 * E
V = 50277
NC = 8
P = 128
JE = E // P          # 8 column-chunks per E-vector
LN_EPS = 1e-5

# per-core head slice (padded)
HV = 6400            # head rows per core (8*6400 = 51200 >= V)
HNT = 13             # 13 tiles of 512 -> 6656 cols of psum output
HPAD = HNT * 512     # 6656

F32 = mybir.dt.float32
BF16 = mybir.dt.bfloat16
WDT = BF16  # weight/matvec-operand dtype
AX = mybir.AxisListType
OP = mybir.AluOpType
AF = mybir.ActivationFunctionType

# column offsets inside the per-layer weight block [128, 13312]
WRW, WKW, WVW, WOW, WFRW, WFKW, WFVW = (
    0, 1024, 2048, 3072, 4096, 5120, 9216)
WCOLS = 13312

# per-layer vec columns (all [128,8] blocks except tf/td shards)
_VNAMES = [
    ("h_k", 8), ("h_v", 8), ("h_r", 8),
    ("om_tmk", 8), ("om_tmv", 8), ("om_tmr", 8),
    ("b_tmk", 8), ("b_tmv", 8), ("b_tmr", 8),
    ("l1w", 8), ("l1b", 8),
    ("h_fk", 8), ("h_fr", 8),
    ("om_ftmk", 8), ("om_ftmr", 8),
    ("b_ftmk", 8), ("b_ftmr", 8),
    ("l2w", 8), ("l2b", 8),
    ("tf", 1), ("td", 1),
]
VOFF = {}
_c = 0
for _n, _w in _VNAMES:
    VOFF[_n] = _c
    _c += _w
VCOLS = _c  # 154

SCOLS = 19  # state in/out per layer: attx(8) ffnx(8) aa bb pp / xn(8) xn2(8) naa nbb npp

RG = [list(range(NC))]


def _build_nc(nl=L):
    nc = bass.Bass(trn_type="TRN2", num_devices=NC)

    wall_d = nc.dram_tensor("wall", [nl, P, WCOLS], WDT, kind="ExternalInput")
    hrhs_d = nc.dram_tensor("hrhs", [HNT, P, 8 * 512], WDT, kind="ExternalInput")
    vecs_d = nc.dram_tensor("vecs", [P, VCOLS * nl], F32, kind="ExternalInput")
    stin_d = nc.dram_tensor("stin", [P, SCOLS * nl], F32, kind="ExternalInput")
    xin_d = nc.dram_tensor("xin", [P, JE], F32, kind="ExternalInput")
    gvec_d = nc.dram_tensor("gvec", [P, 16], F32, kind="ExternalInput")
    mask_d = nc.dram_tensor("mask", [P, JE], F32, kind="ExternalInput")

    logits_d = nc.dram_tensor("logits", [1, HPAD], F32, kind="ExternalOutput")
    stout_d = nc.dram_tensor("stout", [P, SCOLS * nl], F32, kind="ExternalOutput")

    with tile.TileContext(nc, num_cores=NC) as tc:
        with (
            tc.tile_pool(name="res", bufs=1) as res,
            tc.tile_pool(name="wpool", bufs=2) as wpool,
            tc.tile_pool(name="hpool", bufs=2) as hpool,
            tc.tile_pool(name="sc", bufs=2) as sc,
            tc.tile_pool(name="ps8", bufs=2, space="PSUM") as ps8,
            tc.tile_pool(name="psmv", bufs=4, space="PSUM") as psmv,
            tc.tile_pool(name="psln", bufs=1, space="PSUM") as psln,
            tc.tile_pool(name="psbc", bufs=1, space="PSUM") as psbc,
            tc.tile_pool(name="dpool", bufs=2, space="DRAM") as dpool,
        ):
            # ---- resident tiles ----
            vecs = res.tile([P, VCOLS * nl], F32, tag="vecs")
            stin = res.tile([P, SCOLS * nl], F32, tag="stin")
            sout = res.tile([P, SCOLS * nl], F32, tag="sout")
            xin = res.tile([P, JE], F32, tag="xin")
            gvec = res.tile([P, 16], F32, tag="gvec")
            mask = res.tile([P, JE], F32, tag="mask")
            ones_c = res.tile([P, 1], F32, tag="ones_c")
            ones_r = res.tile([1, P], F32, tag="ones_r")
            epsc = res.tile([1, 1], F32, tag="epsc")

            nc.sync.dma_start(vecs[:], vecs_d[:])
            nc.sync.dma_start(stin[:], stin_d[:])
            nc.sync.dma_start(xin[:], xin_d[:])
            nc.sync.dma_start(gvec[:], gvec_d[:])
            nc.sync.dma_start(mask[:], mask_d[:])
            nc.vector.memset(ones_c[:], 1.0)
            nc.vector.memset(ones_r[:], 1.0)
            nc.vector.memset(epsc[:], LN_EPS)

            def vec(l, name, w=8):
                o = l * VCOLS + VOFF[name]
                return vecs[:, o:o + w]

            def ln_z(x_ap, ztag, zdt=F32):
                """z = (x - mean) * rstd, returns [128,8] tile."""
                S = sc.tile([P, 16], F32, tag=ztag + "S")
                nc.vector.tensor_copy(S[:, 0:8], x_ap)
                nc.vector.tensor_mul(S[:, 8:16], x_ap, x_ap)
                lps = psln.tile([1, 16], F32, tag="lnps")
                nc.tensor.matmul(lps[:], ones_c[:], S[:], start=True, stop=True)
                st = sc.tile([1, 2], F32, tag=ztag + "st")
                nc.vector.tensor_reduce(
                    st[:], lps[0:1, :].rearrange("p (g x) -> p g x", g=2),
                    axis=AX.X, op=OP.add)
                nc.vector.tensor_scalar_mul(st[:], st[:], 1.0 / E)
                msq = sc.tile([1, 1], F32, tag=ztag + "m2")
                nc.vector.tensor_mul(msq[:], st[:, 0:1], st[:, 0:1])
                nc.vector.tensor_sub(st[:, 1:2], st[:, 1:2], msq[:])
                nc.scalar.activation(st[:, 1:2], st[:, 1:2], AF.Sqrt,
                                     bias=epsc[:])
                nc.vector.reciprocal(st[:, 1:2], st[:, 1:2])
                bps = psbc.tile([P, 2], F32, tag="bcps")
                nc.tensor.matmul(bps[:], ones_r[:], st[:], start=True, stop=True)
                mb = sc.tile([P, 2], F32, tag=ztag + "mb")
                nc.vector.tensor_copy(mb[:], bps[:])
                z = sc.tile([P, JE], zdt, tag=ztag)
                nc.vector.tensor_scalar(
                    z[:], x_ap, mb[:, 0:1], mb[:, 1:2],
                    op0=OP.subtract, op1=OP.mult)
                return z

            def mix(z, l, hname, gsrc, omname, bname, tag):
                """xk = z*h + (src*om + b); returns [128,8] tile."""
                g = sc.tile([P, JE], F32, tag=tag + "g")
                nc.vector.tensor_mul(g[:], gsrc, vec(l, omname))
                nc.vector.tensor_add(g[:], g[:], vec(l, bname))
                xm = sc.tile([P, JE], WDT, tag=tag)
                nc.vector.tensor_mul(xm[:], z[:], vec(l, hname))
                nc.vector.tensor_add(xm[:], xm[:], g[:])
                return xm

            # ---- ln0 ----
            z0 = ln_z(xin[:], "z0")
            x = sc.tile([P, JE], F32, tag="x", bufs=3)
            nc.vector.tensor_mul(x[:], z0[:], gvec[:, 0:8])
            nc.vector.tensor_add(x[:], x[:], gvec[:, 8:16])

            for l in range(nl):
                so = l * SCOLS
                attx = stin[:, so:so + 8]
                ffnx = stin[:, so + 8:so + 16]
                aa = stin[:, so + 16:so + 17]
                bb = stin[:, so + 17:so + 18]
                pp = stin[:, so + 18:so + 19]

                wsb = wpool.tile([P, WCOLS], WDT, tag="W")
                nc.sync.dma_start(wsb[:], wall_d[l])

                # ---- attention ----
                z1 = ln_z(x[:], "z1")
                # state: xn = z1*l1w + l1b
                nc.vector.tensor_mul(sout[:, so:so + 8], z1[:], vec(l, "l1w"))
                nc.vector.tensor_add(sout[:, so:so + 8], sout[:, so:so + 8],
                                     vec(l, "l1b"))
                xk = mix(z1, l, "h_k", attx, "om_tmk", "b_tmk", "xk")
                xv = mix(z1, l, "h_v", attx, "om_tmv", "b_tmv", "xv")
                xr = mix(z1, l, "h_r", attx, "om_tmr", "b_tmr", "xr")

                kps = psmv.tile([P, 1], F32, tag="mv")
                vps = psmv.tile([P, 1], F32, tag="mv")
                rps = psmv.tile([P, 1], F32, tag="mv")
                for ps, base, xm in ((kps, WKW, xk), (vps, WVW, xv),
                                     (rps, WRW, xr)):
                    for j in range(JE):
                        nc.tensor.matmul(
                            ps[:], wsb[:, base + j * P: base + (j + 1) * P],
                            xm[:, j:j + 1], start=(j == 0), stop=(j == JE - 1))

                k = sc.tile([P, 1], F32, tag="k")
                v = sc.tile([P, 1], F32, tag="v")
                nc.vector.tensor_copy(k[:], kps[:])
                nc.vector.tensor_copy(v[:], vps[:])
                r = sc.tile([P, 1], F32, tag="r")
                nc.scalar.activation(r[:], rps[:], AF.Exp, scale=-1.0)
                nc.vector.tensor_scalar_add(r[:], r[:], 1.0)
                nc.vector.reciprocal(r[:], r[:])

                tf = vec(l, "tf", 1)
                td = vec(l, "td", 1)
                ww = sc.tile([P, 1], F32, tag="ww")
                nc.vector.tensor_scalar_add(ww[:], k[:], tf)
                p_ = sc.tile([P, 1], F32, tag="p_")
                nc.vector.tensor_scalar_max(p_[:], ww[:], pp)
                e1 = sc.tile([P, 1], F32, tag="e1")
                nc.scalar.activation(e1[:], p_[:], AF.Exp, bias=pp, scale=-1.0)
                e2 = sc.tile([P, 1], F32, tag="e2")
                nc.scalar.activation(e2[:], p_[:], AF.Exp, bias=ww[:], scale=-1.0)
                t2 = sc.tile([P, 1], F32, tag="t2")
                nc.vector.tensor_scalar_mul(t2[:], e2[:], v[:])
                a_ = sc.tile([P, 1], F32, tag="a_")
                nc.vector.scalar_tensor_tensor(
                    a_[:], e1[:], aa, t2[:], op0=OP.mult, op1=OP.add)
                b_ = sc.tile([P, 1], F32, tag="b_")
                nc.vector.scalar_tensor_tensor(
                    b_[:], e1[:], bb, e2[:], op0=OP.mult, op1=OP.add)
                rec = sc.tile([P, 1], F32, tag="rec")
                nc.vector.reciprocal(rec[:], b_[:])
                wkv = sc.tile([P, 1], WDT, tag="wkv")
                nc.vector.tensor_scalar(
                    wkv[:], a_[:], r[:], rec[:], op0=OP.mult, op1=OP.mult)

                # state update
                ww2 = sc.tile([P, 1], F32, tag="ww2")
                nc.vector.tensor_scalar_add(ww2[:], pp, td)
                npp = sout[:, so + 18:so + 19]
                nc.vector.tensor_scalar_max(npp, ww2[:], k[:])
                f1 = sc.tile([P, 1], F32, tag="f1")
                nc.scalar.activation(f1[:], npp, AF.Exp, bias=ww2[:], scale=-1.0)
                f2 = sc.tile([P, 1], F32, tag="f2")
                nc.scalar.activation(f2[:], npp, AF.Exp, bias=k[:], scale=-1.0)
                t4 = sc.tile([P, 1], F32, tag="t4")
                nc.vector.tensor_scalar_mul(t4[:], f2[:], v[:])
                nc.vector.scalar_tensor_tensor(
                    sout[:, so + 16:so + 17], f1[:], aa, t4[:],
                    op0=OP.mult, op1=OP.add)
                nc.vector.scalar_tensor_tensor(
                    sout[:, so + 17:so + 18], f1[:], bb, f2[:],
                    op0=OP.mult, op1=OP.add)

                # ow partial (input-sharded): y_t = ow[:,c]_tile_t @ wkv
                yps = ps8.tile([P, JE], F32, tag="yz")
                for t in range(JE):
                    nc.tensor.matmul(
                        yps[:, t:t + 1], wsb[:, WOW + t * P: WOW + (t + 1) * P],
                        wkv[:], start=True, stop=True)
                ysb = sc.tile([P, JE], F32, tag="ysb")
                nc.vector.tensor_copy(ysb[:], yps[:])

                b1i = dpool.tile([P, JE], F32, tag="b1i")
                b1o = dpool.tile([P, JE], F32, tag="b1o")
                nc.gpsimd.dma_start(b1i[:], ysb[:])
                nc.gpsimd.collective_compute(
                    "AllReduce", OP.add, replica_groups=RG,
                    ins=[b1i.opt()], outs=[b1o.opt()])
                yf = sc.tile([P, JE], F32, tag="yf")
                nc.gpsimd.dma_start(yf[:], b1o[:])

                sx = sc.tile([P, JE], F32, tag="sx")
                nc.vector.tensor_add(sx[:], x[:], yf[:])

                # ---- ffn ----
                z2 = ln_z(sx[:], "z2")
                nc.vector.tensor_mul(sout[:, so + 8:so + 16], z2[:], vec(l, "l2w"))
                nc.vector.tensor_add(sout[:, so + 8:so + 16],
                                     sout[:, so + 8:so + 16], vec(l, "l2b"))
                fxk = mix(z2, l, "h_fk", ffnx, "om_ftmk", "b_ftmk", "fxk")
                fxr = mix(z2, l, "h_fr", ffnx, "om_ftmr", "b_ftmr", "fxr")

                fkps = psmv.tile([P, 4], F32, tag="mv")
                for t in range(4):
                    for j in range(JE):
                        nc.tensor.matmul(
                            fkps[:, t:t + 1],
                            wsb[:, WFKW + (t * 8 + j) * P: WFKW + (t * 8 + j + 1) * P],
                            fxk[:, j:j + 1], start=(j == 0), stop=(j == JE - 1))
                fkr = sc.tile([P, 4], F32, tag="fkr")
                nc.vector.tensor_scalar_max(fkr[:], fkps[:], 0.0)
                fks = sc.tile([P, 4], WDT, tag="fks")
                nc.vector.tensor_mul(fks[:], fkr[:], fkr[:])

                frps = psmv.tile([P, 1], F32, tag="mv")
                for j in range(JE):
                    nc.tensor.matmul(
                        frps[:], wsb[:, WFRW + j * P: WFRW + (j + 1) * P],
                        fxr[:, j:j + 1], start=(j == 0), stop=(j == JE - 1))
                fr = sc.tile([P, 1], F32, tag="fr")
                nc.scalar.activation(fr[:], frps[:], AF.Exp, scale=-1.0)
                nc.vector.tensor_scalar_add(fr[:], fr[:], 1.0)
                nc.vector.reciprocal(fr[:], fr[:])

                zps = ps8.tile([P, JE], F32, tag="yz")
                for t in range(JE):
                    for j in range(4):
                        nc.tensor.matmul(
                            zps[:, t:t + 1],
                            wsb[:, WFVW + (t * 4 + j) * P: WFVW + (t * 4 + j + 1) * P],
                            fks[:, j:j + 1], start=(j == 0), stop=(j == 3))

                zfr = sc.tile([P, 16], F32, tag="zfr")
                nc.vector.tensor_copy(zfr[:, 0:8], zps[:])
                nc.vector.tensor_scalar_mul(zfr[:, 8:16], mask[:], fr[:])

                b2i = dpool.tile([P, 16], F32, tag="b2i")
                b2o = dpool.tile([P, 16], F32, tag="b2o")
                nc.gpsimd.dma_start(b2i[:], zfr[:])
                nc.gpsimd.collective_compute(
                    "AllReduce", OP.add, replica_groups=RG,
                    ins=[b2i.opt()], outs=[b2o.opt()])
                yf2 = sc.tile([P, 16], F32, tag="yf2")
                nc.gpsimd.dma_start(yf2[:], b2o[:])

                fz = sc.tile([P, JE], F32, tag="fz")
                nc.vector.tensor_mul(fz[:], yf2[:, 0:8], yf2[:, 8:16])
                xn = sc.tile([P, JE], F32, tag="x", bufs=3)
                nc.vector.tensor_add(xn[:], sx[:], fz[:])
                x = xn

            # ---- head ----
            zo = ln_z(x[:], "zo", zdt=WDT)
            for nt in range(HNT):
                hsb = hpool.tile([P, 8 * 512], WDT, tag="H")
                nc.sync.dma_start(hsb[:], hrhs_d[nt])
                hps = ps8.tile([1, 512], F32, tag="yz")
                for j in range(JE):
                    nc.tensor.matmul(
                        hps[:], zo[:, j:j + 1], hsb[:, j * 512:(j + 1) * 512],
                        start=(j == 0), stop=(j == JE - 1))
                lrow = sc.tile([1, 512], F32, tag="lrow")
                if nt % 2 == 0:
                    nc.vector.tensor_copy(lrow[:], hps[:])
                else:
                    nc.scalar.copy(lrow[:], hps[:])
                nc.sync.dma_start(logits_d[0:1, nt * 512:(nt + 1) * 512], lrow[:])

            nc.sync.dma_start(stout_d[:], sout[:])

    return nc


def _split_multi_waits(nc):
    """Walrus codegen only supports ONE sync-wait per instruction; move
    extra waits onto same-engine NoOps inserted just before."""
    n = 0
    for f in nc.m.functions:
        for blk in f.blocks:
            new = []
            for ins in blk.instructions:
                si = ins.sync_info
                if si is not None and si.on_wait is not None \
                        and len(si.on_wait) > 1:
                    waits = list(si.on_wait)
                    for w in waits[:-1]:
                        nop = mybir.InstNoOp(
                            name=f"I-ws-{n}", ins=[], outs=[])
                        n += 1
                        nop.engine = ins.engine
                        nop.sync_info = mybir.SyncInfo(
                            on_wait=[w], on_update=[])
                        new.append(nop)
                    si.on_wait = [waits[-1]]
                new.append(ins)
            blk.instructions = new


# ---------------------------------------------------------------------------
# host-side prep


def _cols(vv):
    """[1024] -> [128, 8] column layout."""
    return np.ascontiguousarray(vv.reshape(JE, P).T)


def _uncols(m):
    """[128, 8] -> [1024]."""
    return np.ascontiguousarray(m.T).reshape(-1)


def _prep(inp, nl=L):
    f = lambda name: np.asarray(inp[name], np.float32)
    emb = f("emb")
    state = f("state").reshape(nl if False else L, 5, E)[:nl]
    token = int(np.asarray(inp["token"]))

    tmk, tmv, tmr = f("att_tmk")[:nl], f("att_tmv")[:nl], f("att_tmr")[:nl]
    tf, td = f("att_tf")[:nl], f("att_td")[:nl]
    l1w, l1b = f("ln1_w")[:nl], f("ln1_b")[:nl]
    l2w, l2b = f("ln2_w")[:nl], f("ln2_b")[:nl]
    ftmk, ftmr = f("ffn_tmk")[:nl], f("ffn_tmr")[:nl]
    kw, vw, rw, ow = f("att_kw")[:nl], f("att_vw")[:nl], f("att_rw")[:nl], f("att_ow")[:nl]
    fkw, fvw, frw = f("ffn_kw")[:nl], f("ffn_vw")[:nl], f("ffn_rw")[:nl]
    lnow, lnob = f("ln_out_w"), f("ln_out_b")
    head = f("head_w")

    x0 = emb[token]

    # head folding
    hw2 = head * lnow[None, :]
    hw2 = np.concatenate(
        [hw2, np.zeros((NC * HV - V, E), np.float32)], axis=0)
    logit0 = head @ lnob

    in_maps = []
    for c in range(NC):
        r0, r1 = c * P, (c + 1) * P
        wall = np.empty((nl, P, WCOLS), np.float32)
        vecs = np.empty((P, VCOLS * nl), np.float32)
        stin = np.empty((P, SCOLS * nl), np.float32)
        for l in range(nl):
            # output-sharded E x E (stationary layout [k, j, m])
            for base, W in ((WRW, rw[l]), (WKW, kw[l]), (WVW, vw[l])):
                A = W[r0:r1, :]                       # [128, 1024]
                arr = A.T.reshape(JE, P, P).transpose(1, 0, 2)
                wall[l, :, base:base + 1024] = arr.reshape(P, 1024)
            # ow input-sharded
            B = ow[l][:, r0:r1]                       # [1024, 128]
            wall[l, :, WOW:WOW + 1024] = B.T.reshape(P, 1024)
            # frw output-sharded
            A = frw[l][r0:r1, :]
            arr = A.T.reshape(JE, P, P).transpose(1, 0, 2)
            wall[l, :, WFRW:WFRW + 1024] = arr.reshape(P, 1024)
            # fkw output-sharded [512, 1024]
            C = fkw[l][c * 512:(c + 1) * 512, :]
            arr = C.T.reshape(JE, P, 4, P).transpose(1, 2, 0, 3)
            wall[l, :, WFKW:WFKW + 4096] = arr.reshape(P, 4096)
            # fvw input-sharded [1024, 512]
            D = fvw[l][:, c * 512:(c + 1) * 512]
            arr = D.T.reshape(4, P, JE, P).transpose(1, 2, 0, 3)
            wall[l, :, WFVW:WFVW + 4096] = arr.reshape(P, 4096)

            o = l * VCOLS
            pairs = {
                "h_k": l1w[l] * tmk[l], "h_v": l1w[l] * tmv[l],
                "h_r": l1w[l] * tmr[l],
                "om_tmk": 1.0 - tmk[l], "om_tmv": 1.0 - tmv[l],
                "om_tmr": 1.0 - tmr[l],
                "b_tmk": l1b[l] * tmk[l], "b_tmv": l1b[l] * tmv[l],
                "b_tmr": l1b[l] * tmr[l],
                "l1w": l1w[l], "l1b": l1b[l],
                "h_fk": l2w[l] * ftmk[l], "h_fr": l2w[l] * ftmr[l],
                "om_ftmk": 1.0 - ftmk[l], "om_ftmr": 1.0 - ftmr[l],
                "b_ftmk": l2b[l] * ftmk[l], "b_ftmr": l2b[l] * ftmr[l],
                "l2w": l2w[l], "l2b": l2b[l],
            }
            for nme, val in pairs.items():
                vecs[:, o + VOFF[nme]:o + VOFF[nme] + 8] = _cols(val)
            vecs[:, o + VOFF["tf"]] = tf[l, r0:r1]
            vecs[:, o + VOFF["td"]] = td[l, r0:r1]

            so = l * SCOLS
            stin[:, so:so + 8] = _cols(state[l, 1])       # att_x
            stin[:, so + 8:so + 16] = _cols(state[l, 0])  # ffn_x
            stin[:, so + 16] = state[l, 2, r0:r1]         # aa
            stin[:, so + 17] = state[l, 3, r0:r1]         # bb
            stin[:, so + 18] = state[l, 4, r0:r1]         # pp

        Hc = hw2[c * HV:(c + 1) * HV]                     # [6400, 1024]
        Hp = np.concatenate(
            [Hc, np.zeros((HPAD - HV, E), np.float32)], axis=0)
        hrhs = Hp.T.reshape(JE, P, HNT, 512).transpose(2, 1, 0, 3)
        hrhs = np.ascontiguousarray(hrhs.reshape(HNT, P, 8 * 512))

        gvec = np.empty((P, 16), np.float32)
        gvec[:, 0:8] = _cols(f("ln0_w"))
        gvec[:, 8:16] = _cols(f("ln0_b"))

        msk = np.zeros((P, JE), np.float32)
        msk[:, c] = 1.0

        npw = ml_dtypes.bfloat16 if WDT == BF16 else np.float32
        in_maps.append({
            "wall": np.ascontiguousarray(wall).astype(npw),
            "hrhs": hrhs.astype(npw),
            "vecs": np.ascontiguousarray(vecs),
            "stin": np.ascontiguousarray(stin),
            "xin": _cols(x0),
            "gvec": gvec,
            "mask": msk,
        })
    return in_maps, logit0


def _assemble(results, logit0, nl=L):
    logits = np.concatenate(
        [results[c]["logits"][0, :HV] for c in range(NC)])[:V] + logit0
    st = results[0]["stout"]
    new_state = np.empty((5 * nl, E), np.float32)
    for l in range(nl):
        so = l * SCOLS
        new_state[5 * l + 0] = _uncols(st[:, so + 8:so + 16])   # ffn_x = xn2
        new_state[5 * l + 1] = _uncols(st[:, so:so + 8])        # att_x = xn
        for c in range(NC):
            stc = results[c]["stout"]
            new_state[5 * l + 2, c * P:(c + 1) * P] = stc[:, so + 16]
            new_state[5 * l + 3, c * P:(c + 1) * P] = stc[:, so + 17]
            new_state[5 * l + 4, c * P:(c + 1) * P] = stc[:, so + 18]
    return logits.astype(np.float32), new_state


_NC_CACHE = {}


def get_nc(nl=L):
    """Build + legalize for the hardware path (sim chokes on the NoOps)."""
    if nl not in _NC_CACHE:
        nc = _build_nc(nl)
        _split_multi_waits(nc)
        _NC_CACHE[nl] = nc
    return _NC_CACHE[nl]


def kernel(**inputs):
    from concourse.bass_utils import run_bass_kernel_spmd

    nc = get_nc(L)
    in_maps, logit0 = _prep(inputs, L)
    res = run_bass_kernel_spmd(nc, in_maps, core_ids=list(range(NC)))
    return _assemble(res.results, logit0, L)
